# revision 1
# baseline (speedup 1.0000x reference)
"""Trainium2 Bass kernel for nn_BlastLinear (block low-rank linear layer).

Math (reference):
  y[q,n,r] = sum_c x[n, q*1024+c] * C[q,r,c]          (mm1, per input block q)
  z[p,n,r] = sum_q D[p,q,r] * y[q,n,r]                (tiny mix over q)
  o[p,n,j] = sum_r z[p,n,r] * B[p,j,r]                (mm2, per output block p)
  out[n, p*1024+j] = o[p,n,j] + bias[p*1024+j]

Sharding: pure data-parallel over the 8192 tokens -> 1024 tokens per core,
weights replicated, no collectives.

Precision: the PE's fast fp32 path (float32r) keeps only 12 significand
bits, so every operand A is split on the host (or on-chip for z) into
A = Ah + Al with both parts exactly f32r-representable, and each matmul
runs three f32r passes accumulating in the same PSUM group:
  A@X ~= Ah@Xh + Ah@Xl + Al@Xh      (drops only Al@Xl ~ 2^-24)
This is 3 cycles/row vs 4 for the native fp32 path, with ~1e-7 rel error.

Per-core pipeline (chunk = 512 tokens, 2 chunks):
  mm1:  psum y^T[q,rt] [128r x 512n] += 3-pass over k of ct^T @ xt  (PE)
  mix:  z[p,rt] = sum_q D[p,q,rt-slice] * y^T[q,rt]   (DVE fused mul-add,
        reads y straight from PSUM, accumulates fp32 in SBUF)
  split: zh = f32r(z), zl = z - zh                    (DVE)
  mm2:  psum o[mt,oc] = 3-pass over rt of z^T @ bt    (PE)
  out:  DVE drains psum -> SBUF fused with the bias add
        (bias pre-broadcast once into resident [128,512] tiles), DMA out.
ct_hi stays resident in SBUF; ct_lo / bt_hi / bt_lo stream per chunk.
TimelineSim (the CoreSim cost model): ~366 us/core, PE-bound at 92% with
PE busy at the 3-pass floor; modeled DMA ~295 us (~96 MiB; ct_lo ships as bf16 for the third mm1 pass, with a bf16 twin of x_hi cast on GPSIMD).
"""

import numpy as np

import concourse.mybir as mybir
import concourse.tile as tile
from concourse import bacc
from concourse.bass_utils import run_bass_kernel_spmd

N_CORES = 8
IN_F = 4096
OUT_F = 4096
P = 4
Q = 4
R = 512
CB = IN_F // Q        # 1024 input features per q block
OB = OUT_F // P       # 1024 output features per p block
N_TOK = 4 * 2048      # 8192 total tokens
N_CORE = N_TOK // N_CORES   # 1024 tokens per core

CHUNK = 512           # tokens per pipeline chunk
KT1 = CB // 128       # 8 contraction tiles per q in mm1
RT = R // 128         # 4 rank partition tiles
KB = 2                # k-tiles per x DMA batch

F32 = mybir.dt.float32
F32R = mybir.dt.float32r
BF16 = mybir.dt.bfloat16
MULT = mybir.AluOpType.mult
ADD = mybir.AluOpType.add
SUB = mybir.AluOpType.subtract

_cached_nc = None


def round_f32r(a):
    """Round fp32 array to f32r (12-bit significand), RTNE at bit 12."""
    u = np.ascontiguousarray(a, dtype=np.float32).view(np.uint32)
    lsb = (u >> 12) & np.uint32(1)
    u2 = (u + np.uint32(0x7FF) + lsb) & np.uint32(0xFFFFF000)
    return u2.view(np.float32)


def _build(n_core=N_CORE, chunk=CHUNK):
    nc = bacc.Bacc("TRN2", target_bir_lowering=False, debug=False,
                   enable_asserts=False)

    def din(name, shape, dtype=F32R):
        return nc.dram_tensor(name, shape, dtype, kind="ExternalInput").ap()

    xth = din("xth", [IN_F, n_core])
    xtl = din("xtl", [IN_F, n_core])
    cth = din("cth", [IN_F, R])
    ctl = din("ctl", [IN_F, R], BF16)
    bth = din("bth", [P * R, OB])
    btl = din("btl", [P * R, OB])
    dr = din("dr", [R, P * Q], F32)
    bias2 = din("bias2", [2, OUT_F])   # row 0: hi, row 1: lo
    onesd = din("onesd", [2, 128])
    out = nc.dram_tensor("out", [n_core, OUT_F], F32,
                         kind="ExternalOutput").ap()

    n_chunks = n_core // chunk
    MT = chunk // 128     # mm2 token tiles per chunk
    OC = OB // 512        # output free-dim chunks per p

    with tile.TileContext(nc) as tc:
        with (
            tc.tile_pool(name="const", bufs=1) as cpool,
            tc.tile_pool(name="ctlp", bufs=2) as ctlpool,
            tc.tile_pool(name="xp", bufs=3) as xpool,
            tc.tile_pool(name="btp", bufs=6) as btpool,
            tc.tile_pool(name="zp", bufs=16) as zpool,
            tc.tile_pool(name="zhp", bufs=7) as zhpool,
            tc.tile_pool(name="outp", bufs=3) as outpool,
            tc.tile_pool(name="biasp", bufs=1) as biaspool,
            tc.tile_pool(name="yps", bufs=6, space="PSUM") as ypool,
            tc.tile_pool(name="ops", bufs=2, space="PSUM") as opool,
        ):
            # cth_sb[p, q*8+k, r]: C^T_hi tile rows c = (q*8+k)*128 + p
            # DMA'd piecewise inside chunk 0's loop so matmuls start early.
            cth_sb = cpool.tile([128, IN_F // 128, R], F32R)
            cth3 = cth.rearrange("(t p) r -> p t r", p=128)
            # d_sb[p_, rt, p*4 + q] = D[p, q, rt*128 + p_]
            # (DMA'd after the first x tiles; see emit_mm1 j=0/q=0)
            d_sb = cpool.tile([128, RT, P * Q], F32)
            ones_sb = cpool.tile([2, 128], F32R)

            z = {}
            zsplit = {}
            bias_bc = {}

            def emit_bias_bc():
                # broadcast bias (hi+lo, exact) to [128, 512] tiles once;
                # mm2 then folds the add into the DVE psum drain
                for p in range(P):
                    for oc in range(OC):
                        off = p * OB + oc * 512
                        b2_t = biaspool.tile([2, 512], F32R, tag="bi2",
                                             name=f"bi2_{p}_{oc}")
                        nc.sync.dma_start(b2_t[:], bias2[0:2, off:off + 512])
                        bps = opool.tile([128, 512], F32, tag="o",
                                         name=f"bps_{p}_{oc}")
                        nc.tensor.matmul(ops := bps[:], lhsT=ones_sb[:],
                                         rhs=b2_t[:], start=True, stop=True)
                        bc = cpool.tile([128, 512], F32, tag=f"bc_{p}_{oc}",
                                        name=f"bc_{p}_{oc}")
                        nc.scalar.copy(bc[:], ops)
                        bias_bc[(p, oc)] = bc

            def emit_zsplit(j, p, rt):
                # cast on ACT (idle) keeps the DVE critical chain short;
                # the subtract stays on DVE.
                zt = z.pop((j, p, rt))
                zh_t = zhpool.tile([128, chunk], F32R, tag="zh",
                                   name=f"zh_{j}_{p}_{rt}")
                nc.scalar.copy(zh_t[:], zt[:])
                zl_t = zhpool.tile([128, chunk], F32R, tag="zl",
                                   name=f"zl_{j}_{p}_{rt}")
                nc.gpsimd.tensor_tensor(
                    zl_t[:], zt[:], zh_t[:].bitcast(F32), op=SUB)
                zsplit[(j, p, rt)] = (zh_t, zl_t)

            def emit_bt_dma(j, p, oc):
                off = p * OB + oc * 512
                hts, lts = [], []
                for rt in range(RT):
                    rb = p * R + rt * 128
                    bth_t = btpool.tile([128, 512], F32R, tag="bth",
                                        name=f"bth_{j}_{p}_{oc}_{rt}")
                    nc.sync.dma_start(
                        bth_t[:], bth[rb:rb + 128, oc * 512:(oc + 1) * 512])
                    hts.append(bth_t)
                    btl_t = btpool.tile([128, 512], F32R, tag="btl",
                                        name=f"btl_{j}_{p}_{oc}_{rt}")
                    nc.sync.dma_start(
                        btl_t[:], btl[rb:rb + 128, oc * 512:(oc + 1) * 512])
                    lts.append(btl_t)
                return hts, lts

            bt_pre = {}

            def emit_mm1(j):
                for q in range(Q):
                    if j == 0 and q > 0:
                        qs = slice(q * KT1, (q + 1) * KT1)
                        nc.sync.dma_start(cth_sb[:, qs, :], cth3[:, qs, :])
                    ys = [
                        ypool.tile([128, chunk], F32, tag="y",
                                   name=f"y_{j}_{q}_{rt}")
                        for rt in range(RT)
                    ]
                    for kb in range(KT1 // KB):
                        if j == 0 and q == 0:
                            # q0's cth piece rides just ahead of its own
                            # kb's x tiles, so the first matmul waits on
                            # ~1 MiB of DMA, not the whole 2 MiB of q0
                            hs = slice(kb * KB, (kb + 1) * KB)
                            nc.sync.dma_start(cth_sb[:, hs, :],
                                              cth3[:, hs, :])
                        if j == 0 and q == 0 and kb == 1:
                            nc.sync.dma_start(
                                d_sb[:],
                                dr.rearrange("(t p) s -> p t s", p=128))
                            nc.sync.dma_start(ones_sb[:], onesd[:])
                        if j == 0 and q == 1 and kb == 0:
                            emit_bias_bc()
                        if q == Q - 1 and kb == 2:
                            # prefetch first mm2 weight group late in q3,
                            # after q3's own x DMAs are underway
                            bt_pre[(j, 0, 0)] = emit_bt_dma(j, 0, 0)
                        base = (q * KT1 + kb * KB) * 128
                        xh_t = xpool.tile([128, KB, chunk], F32R, tag="xh",
                                          name=f"xh_{j}_{q}_{kb}")
                        xl_t = xpool.tile([128, KB, chunk], F32R, tag="xl",
                                          name=f"xl_{j}_{q}_{kb}")
                        first = j == 0 and q == 0 and kb == 0
                        for src_d, t in ((xth, xh_t), (xtl, xl_t)):
                            # per-k pieces at kernel start so the first
                            # matmul waits on ~512 KiB, not the full batch
                            pieces = KB if first else 1
                            for pc in range(pieces):
                                w = KB // pieces
                                nc.sync.dma_start(
                                    t[:, pc * w:(pc + 1) * w, :],
                                    src_d[base + pc * w * 128:
                                          base + (pc + 1) * w * 128,
                                          j * chunk:(j + 1) * chunk]
                                    .rearrange("(t p) n -> p t n", p=128))
                        ctl_t = ctlpool.tile([128, KB, R], BF16, tag="ctl",
                                             name=f"ctl_{j}_{q}_{kb}")
                        nc.sync.dma_start(
                            ctl_t[:],
                            ctl[base:base + KB * 128, :]
                            .rearrange("(t p) r -> p t r", p=128))
                        # bf16 twin of xh for the bf16 lo-weight pass
                        xhb_t = xpool.tile([128, KB, chunk], BF16, tag="xhb",
                                           name=f"xhb_{j}_{q}_{kb}", bufs=2)
                        nc.gpsimd.tensor_copy(
                            xhb_t[:], xh_t[:].bitcast(F32))
                        for rt in range(RT):
                            for kk in range(KB):
                                k = kb * KB + kk
                                hi_w = cth_sb[:, q * KT1 + k,
                                              rt * 128:(rt + 1) * 128]
                                lo_w = ctl_t[:, kk, rt * 128:(rt + 1) * 128]
                                nc.tensor.matmul(
                                    ys[rt][:], lhsT=hi_w, rhs=xh_t[:, kk, :],
                                    start=(k == 0), stop=False)
                                nc.tensor.matmul(
                                    ys[rt][:], lhsT=hi_w, rhs=xl_t[:, kk, :],
                                    start=False, stop=False)
                                nc.tensor.matmul(
                                    ys[rt][:], lhsT=lo_w, rhs=xhb_t[:, kk, :],
                                    start=False, stop=(k == KT1 - 1))
                    # rt-major frees each y PSUM bank after 4 ops; on the
                    # last q, split z into f32r hi/lo right after its final
                    # accumulation so mm2 isn't gated on a DVE tail.
                    for rt in range(RT):
                        for p in range(P):
                            col = p * Q + q
                            dcol = d_sb[:, rt, col:col + 1]
                            if q == 0:
                                zt = zpool.tile([128, chunk], F32, tag="z",
                                                name=f"z_{j}_{p}_{rt}")
                                z[(j, p, rt)] = zt
                                nc.vector.tensor_scalar_mul(
                                    zt[:], ys[rt][:], dcol)
                            else:
                                zt = z[(j, p, rt)]
                                nc.vector.scalar_tensor_tensor(
                                    zt[:], ys[rt][:], dcol, zt[:],
                                    op0=MULT, op1=ADD)
                            if q == Q - 1 and p == 0:
                                # eager split for p0 only: it gates mm2 start
                                emit_zsplit(j, p, rt)

            def emit_mm2(j):
                for p in range(P):
                    for rt in range(RT):
                        if (j, p, rt) not in zsplit:
                            emit_zsplit(j, p, rt)
                    zh = {rt: zsplit[(j, p, rt)][0] for rt in range(RT)}
                    zl = {rt: zsplit[(j, p, rt)][1] for rt in range(RT)}
                    for oc in range(OC):
                        off = p * OB + oc * 512
                        if (j, p, oc) in bt_pre:
                            bth_ts, btl_ts = bt_pre.pop((j, p, oc))
                        else:
                            bth_ts, btl_ts = emit_bt_dma(j, p, oc)
                        for mt in range(MT):
                            ops = opool.tile([128, 512], F32, tag="o",
                                             name=f"o_{j}_{p}_{oc}_{mt}")
                            ms = slice(mt * 128, (mt + 1) * 128)
                            for rt in range(RT):
                                nc.tensor.matmul(
                                    ops[:], lhsT=zh[rt][:, ms],
                                    rhs=bth_ts[rt][:],
                                    start=(rt == 0), stop=False)
                                nc.tensor.matmul(
                                    ops[:], lhsT=zh[rt][:, ms],
                                    rhs=btl_ts[rt][:],
                                    start=False, stop=False)
                                nc.tensor.matmul(
                                    ops[:], lhsT=zl[rt][:, ms],
                                    rhs=bth_ts[rt][:],
                                    start=False, stop=(rt == RT - 1))
                            ot = outpool.tile([128, 512], F32, tag="ot",
                                              name=f"ot_{j}_{p}_{oc}_{mt}")
                            nc.vector.tensor_tensor(
                                ot[:], ops[:], bias_bc[(p, oc)][:], op=ADD)
                            nc.sync.dma_start(
                                out[j * chunk + mt * 128:
                                    j * chunk + (mt + 1) * 128,
                                    off:off + 512],
                                ot[:])

            for j in range(n_chunks):
                emit_mm1(j)
                emit_mm2(j)

    nc.compile()
    return nc


def _prep_in_maps(x, B, C, D, bias):
    x2 = np.ascontiguousarray(
        np.asarray(x, dtype=np.float32).reshape(N_TOK, IN_F))
    CT = np.ascontiguousarray(
        np.asarray(C, dtype=np.float32).transpose(0, 2, 1).reshape(IN_F, R))
    BT = np.ascontiguousarray(
        np.asarray(B, dtype=np.float32).transpose(0, 2, 1).reshape(P * R, OB))
    DR = np.ascontiguousarray(
        np.asarray(D, dtype=np.float32).transpose(2, 0, 1).reshape(R, P * Q))
    bias2 = np.ascontiguousarray(
        np.asarray(bias, dtype=np.float32).reshape(1, OUT_F))

    import ml_dtypes
    CTH = round_f32r(CT)
    CTL = np.ascontiguousarray((CT - CTH).astype(ml_dtypes.bfloat16))
    BTH = round_f32r(BT)
    BTL = np.ascontiguousarray(BT - BTH)
    BIH = round_f32r(bias2)
    BI2 = np.ascontiguousarray(
        np.concatenate([BIH, bias2 - BIH], axis=0))
    ONES = np.ones((2, 128), dtype=np.float32)

    in_maps = []
    for c in range(N_CORES):
        xt = np.ascontiguousarray(x2[c * N_CORE:(c + 1) * N_CORE].T)
        xh = round_f32r(xt)
        xl = np.ascontiguousarray(xt - xh)
        in_maps.append({
            "xth": xh, "xtl": xl, "cth": CTH, "ctl": CTL,
            "bth": BTH, "btl": BTL, "dr": DR,
            "bias2": BI2, "onesd": ONES,
        })
    return in_maps


def _run(in_maps, trace=False):
    global _cached_nc
    if _cached_nc is None:
        _cached_nc = _build()
    import time
    for attempt in range(3):
        try:
            return run_bass_kernel_spmd(
                _cached_nc, in_maps, list(range(N_CORES)), trace=trace)
        except Exception:
            # transient device errors (e.g. NRT_EXEC_UNIT_UNRECOVERABLE
            # from a previously wedged core) usually clear on retry
            if attempt == 2:
                raise
            time.sleep(5.0 * (attempt + 1))


def kernel(x, B, C, D, bias):
    lead = np.asarray(x).shape[:-1]
    res = _run(_prep_in_maps(x, B, C, D, bias))
    outs = [res.results[c]["out"] for c in range(N_CORES)]
    return np.concatenate(outs, axis=0).reshape(*lead, OUT_F)



# revision 6
# speedup vs baseline: 2.9085x; 2.9085x over previous
"""Trainium2 Bass kernel for nn_BlastLinear (block low-rank linear layer).

Math (reference):
  y[q,n,r] = sum_c x[n, q*1024+c] * C[q,r,c]          (mm1, per input block q)
  z[p,n,r] = sum_q D[p,q,r] * y[q,n,r]                (tiny mix over q)
  o[p,n,j] = sum_r z[p,n,r] * B[p,j,r]                (mm2, per output block p)
  out[n, p*1024+j] = o[p,n,j] + bias[p*1024+j]

Sharding: pure data-parallel over the 8192 tokens -> 1024 tokens per core,
weights replicated, no collectives.

Precision: single-pass bf16 matmuls (PSUM accumulates fp32). The harness
gate is rel_err < 2e-2; bf16 rounding of x/C/B plus the bf16 y/z
carries lands around ~5e-3 worst-case relative error - comfortably in.
This is 1 PE pass per matmul vs 4 for native fp32 (3 for split f32r),
putting PE at the 512-matmul floor: 512 x 512cols x 0.4167ns = 109 us.

Per-core structure (chunk = 512 tokens, 2 chunks, PE order
mm1(c0), mm1(c1), mm2(c0), mm2(c1) so mm2 never waits on the mix):
  mm1:  psum y[q,rt] [128r x 512n] += ct^T @ xt   (PE, 8 k-tiles per q)
  ycp:  yb = bf16(y)  PSUM->SBUF on ACT           (keeps DVE off PSUM)
  mix:  zb[p,rt] += D[p,q,rt]*yb[q,rt]            (DVE+Pool split, bf16)
  mm2:  psum oT[ot] [128o x 512n] += bt^T @ zb    (PE, 4 rt-tiles)
  drain: ACT activation Identity with per-partition bias AP: fuses the
        bias add into the PSUM drain; out ships bf16 TRANSPOSED
        [OUT_F, n_core] and the host un-transposes + casts to f32.
Weights (C 4MiB, B 4MiB bf16) stay resident in SBUF; x streams per
k-batch; total DMA ~24 MiB ~ 70us, hidden under PE.
"""

import numpy as np

import concourse.mybir as mybir
import concourse.tile as tile
from concourse import bacc
from concourse.bass_utils import run_bass_kernel_spmd

N_CORES = 8
IN_F = 4096
OUT_F = 4096
P = 4
Q = 4
R = 512
CB = IN_F // Q        # 1024 input features per q block
OB = OUT_F // P       # 1024 output features per p block
N_TOK = 4 * 2048      # 8192 total tokens
N_CORE = N_TOK // N_CORES   # 1024 tokens per core

CHUNK = 512           # tokens per pipeline chunk
KT1 = CB // 128       # 8 contraction tiles per q in mm1
RT = R // 128         # 4 rank partition tiles
KB = 2                # k-tiles per x DMA batch
OT = OB // 128        # 8 output-feature tiles per p

F32 = mybir.dt.float32
BF16 = mybir.dt.bfloat16
MULT = mybir.AluOpType.mult
ADD = mybir.AluOpType.add
IDENT = mybir.ActivationFunctionType.Identity

_cached_nc = None


def _build(n_core=N_CORE, chunk=CHUNK):
    nc = bacc.Bacc("TRN2", target_bir_lowering=False, debug=False,
                   enable_asserts=False)

    def din(name, shape, dtype):
        return nc.dram_tensor(name, shape, dtype, kind="ExternalInput").ap()

    xt = din("xt", [IN_F, n_core], BF16)
    ct = din("ct", [IN_F, R], BF16)
    bt = din("bt", [P * R, OB], BF16)
    dr = din("dr", [R, P * Q], F32)
    biasd = din("biasd", [OUT_F], F32)
    outT = nc.dram_tensor("outT", [OUT_F, n_core], BF16,
                          kind="ExternalOutput").ap()

    n_chunks = n_core // chunk

    with tile.TileContext(nc) as tc:
        with (
            tc.tile_pool(name="const", bufs=1) as cpool,
            tc.tile_pool(name="xp", bufs=4) as xpool,
            tc.tile_pool(name="ybp", bufs=16) as ybpool,
            tc.tile_pool(name="zbp", bufs=2 * P * RT * n_chunks) as zbpool,
            tc.tile_pool(name="outp", bufs=4) as outpool,
            tc.tile_pool(name="yps", bufs=6, space="PSUM") as ypool,
            tc.tile_pool(name="ops", bufs=2, space="PSUM") as opool,
        ):
            # ct_sb[c_, q*8+k, r]: C^T tile rows c = (q*8+k)*128 + c_
            ct_sb = cpool.tile([128, IN_F // 128, R], BF16)
            ct3 = ct.rearrange("(t p) r -> p t r", p=128)
            # bt_sb[r_, p*4+rt, o]: B^T tile rows r = (p*4+rt)*128 + r_
            bt_sb = cpool.tile([128, (P * R) // 128, OB], BF16)
            bt3 = bt.rearrange("(t p) o -> p t o", p=128)
            # d_sb[r_, rt, p*4+q] = D[p, q, rt*128 + r_]
            d_sb = cpool.tile([128, RT, P * Q], F32)
            # bias_sb[o_, g] = bias[g*128 + o_]  (g = p*OT + ot)
            bias_sb = cpool.tile([128, OUT_F // 128], F32)

            zb = {}

            def emit_mm1(j):
                for q in range(Q):
                    ys = [
                        ypool.tile([128, chunk], F32, tag="y",
                                   name=f"y_{j}_{q}_{rt}")
                        for rt in range(RT)
                    ]
                    for kb in range(KT1 // KB):
                        base_t = q * KT1 + kb * KB
                        if j == 0:
                            # C rides piecewise just ahead of its x tiles
                            hs = slice(base_t, base_t + KB)
                            nc.sync.dma_start(ct_sb[:, hs, :], ct3[:, hs, :])
                        if j == 0 and q == 0 and kb == 1:
                            nc.sync.dma_start(
                                d_sb[:],
                                dr.rearrange("(t p) s -> p t s", p=128))
                            nc.sync.dma_start(
                                bias_sb[:],
                                biasd.rearrange("(t p) -> p t", p=128))
                        if j == n_chunks - 1 and kb == 2:
                            # B prefetch spread over chunk 1's mm1 (DMA is
                            # idle-ish here; B is needed only at mm2 c0)
                            bs = slice(q * RT, (q + 1) * RT)
                            nc.sync.dma_start(bt_sb[:, bs, :], bt3[:, bs, :])
                        x_t = xpool.tile([128, KB, chunk], BF16, tag="x",
                                         name=f"x_{j}_{q}_{kb}")
                        first = j == 0 and q == 0 and kb == 0
                        # per-k pieces at kernel start so the first matmul
                        # waits on ~128 KiB, not the full batch
                        pieces = KB if first else 1
                        for pc in range(pieces):
                            w = KB // pieces
                            nc.sync.dma_start(
                                x_t[:, pc * w:(pc + 1) * w, :],
                                xt[(base_t + pc * w) * 128:
                                   (base_t + (pc + 1) * w) * 128,
                                   j * chunk:(j + 1) * chunk]
                                .rearrange("(t p) n -> p t n", p=128))
                        for kk in range(KB):
                            k = kb * KB + kk
                            for rt in range(RT):
                                nc.tensor.matmul(
                                    ys[rt][:],
                                    lhsT=ct_sb[:, base_t + kk,
                                               rt * 128:(rt + 1) * 128],
                                    rhs=x_t[:, kk, :],
                                    start=(k == 0), stop=(k == KT1 - 1))
                    # y -> SBUF bf16 on ACT, then the D-mix on DVE+Pool
                    ybs = []
                    for rt in range(RT):
                        yb_t = ybpool.tile([128, chunk], BF16, tag="yb",
                                           name=f"yb_{j}_{q}_{rt}")
                        nc.scalar.copy(yb_t[:], ys[rt][:])
                        ybs.append(yb_t)
                    # TensorScalarPtr is DVE-only on real HW (walrus rejects
                    # it on Pool). q0's mul runs at the DVE 4x bf16 rate;
                    # the q>0 fused mul-accumulates run at 1x.
                    for rt in range(RT):
                        for p in range(P):
                            col = p * Q + q
                            dcol = d_sb[:, rt, col:col + 1]
                            if q == 0:
                                zt = zbpool.tile([128, chunk], BF16, tag="zb",
                                                 name=f"zb_{j}_{p}_{rt}")
                                zb[(j, p, rt)] = zt
                                nc.vector.tensor_scalar_mul(
                                    zt[:], ybs[rt][:], dcol)
                            else:
                                zt = zb[(j, p, rt)]
                                nc.vector.scalar_tensor_tensor(
                                    zt[:], ybs[rt][:], dcol, zt[:],
                                    op0=MULT, op1=ADD)

            def emit_mm2(j):
                for p in range(P):
                    for ot in range(OT):
                        g = p * OT + ot
                        ops = opool.tile([128, chunk], F32, tag="o",
                                         name=f"o_{j}_{g}")
                        for rt in range(RT):
                            nc.tensor.matmul(
                                ops[:],
                                lhsT=bt_sb[:, p * RT + rt,
                                           ot * 128:(ot + 1) * 128],
                                rhs=zb[(j, p, rt)][:],
                                start=(rt == 0), stop=(rt == RT - 1))
                        ot_sb = outpool.tile([128, chunk], BF16, tag="ot",
                                             name=f"ot_{j}_{g}")
                        # fused PSUM drain + per-partition bias add on ACT
                        nc.scalar.activation(
                            ot_sb[:], ops[:], IDENT,
                            bias=bias_sb[:, g:g + 1], scale=1.0)
                        nc.sync.dma_start(
                            outT[g * 128:(g + 1) * 128,
                                 j * chunk:(j + 1) * chunk],
                            ot_sb[:])

            for j in range(n_chunks):
                emit_mm1(j)
            for j in range(n_chunks):
                emit_mm2(j)

    nc.compile()
    return nc


def _prep_in_maps(x, B, C, D, bias):
    import ml_dtypes
    x2 = np.asarray(x, dtype=np.float32).reshape(N_TOK, IN_F)
    CT = np.ascontiguousarray(
        np.asarray(C, dtype=np.float32).transpose(0, 2, 1).reshape(IN_F, R)
    ).astype(ml_dtypes.bfloat16)
    BT = np.ascontiguousarray(
        np.asarray(B, dtype=np.float32).transpose(0, 2, 1).reshape(P * R, OB)
    ).astype(ml_dtypes.bfloat16)
    DR = np.ascontiguousarray(
        np.asarray(D, dtype=np.float32).transpose(2, 0, 1).reshape(R, P * Q))
    BI = np.ascontiguousarray(np.asarray(bias, dtype=np.float32))

    in_maps = []
    for c in range(N_CORES):
        xtc = np.ascontiguousarray(
            x2[c * N_CORE:(c + 1) * N_CORE].T).astype(ml_dtypes.bfloat16)
        in_maps.append({
            "xt": xtc, "ct": CT, "bt": BT, "dr": DR, "biasd": BI,
        })
    return in_maps


def _run(in_maps, trace=False):
    global _cached_nc
    if _cached_nc is None:
        _cached_nc = _build()
    import time
    for attempt in range(3):
        try:
            return run_bass_kernel_spmd(
                _cached_nc, in_maps, list(range(N_CORES)), trace=trace)
        except Exception:
            # transient device errors (e.g. NRT_EXEC_UNIT_UNRECOVERABLE
            # from a previously wedged core) usually clear on retry
            if attempt == 2:
                raise
            time.sleep(5.0 * (attempt + 1))


def kernel(x, B, C, D, bias):
    lead = np.asarray(x).shape[:-1]
    res = _run(_prep_in_maps(x, B, C, D, bias))
    outs = [
        np.asarray(res.results[c]["outT"]).astype(np.float32).T
        for c in range(N_CORES)
    ]
    return np.concatenate(outs, axis=0).reshape(*lead, OUT_F)


# revision 31
# speedup vs baseline: 2.9519x; 1.0149x over previous
"""Trainium2 Bass kernel for nn_BlastLinear (block low-rank linear layer).

Math (reference):
  y[q,n,r] = sum_c x[n, q*1024+c] * C[q,r,c]          (mm1, per input block q)
  z[p,n,r] = sum_q D[p,q,r] * y[q,n,r]                (tiny mix over q)
  o[p,n,j] = sum_r z[p,n,r] * B[p,j,r]                (mm2, per output block p)
  out[n, p*1024+j] = o[p,n,j] + bias[p*1024+j]

Sharding: pure data-parallel over the 8192 tokens -> 1024 tokens per core,
weights replicated, no collectives.

Precision: single-pass bf16 matmuls (PSUM accumulates fp32). The harness
gate is rel_err < 2e-2; bf16 rounding of x/C/B plus the bf16 y/z
carries lands around ~5e-3 worst-case relative error - comfortably in.
This is 1 PE pass per matmul vs 4 for native fp32 (3 for split f32r),
putting PE at the 512-matmul floor: 512 x 512cols x 0.4167ns = 109 us.

Per-core structure (chunk = 512 tokens, 2 chunks, PE order
mm1(c0), mm1(c1), mm2(c0), mm2(c1) so mm2 never waits on the mix):
  mm1:  psum y[q,rt] [128r x 512n] += ct^T @ xt   (PE, 8 k-tiles per q)
  ycp:  yb = bf16(y)  PSUM->SBUF on ACT           (keeps DVE off PSUM)
  mix:  zb[p,rt] += D[p,q,rt]*yb[q,rt]            (DVE+Pool split, bf16)
  mm2:  psum oT[ot] [128o x 512n] += bt^T @ zb    (PE, 4 rt-tiles)
  drain: ACT activation Identity with per-partition bias AP: fuses the
        bias add into the PSUM drain; out ships bf16 TRANSPOSED
        [OUT_F, n_core] and the host un-transposes + casts to f32.
Weights (C 4MiB, B 4MiB bf16) stay resident in SBUF; x streams per
k-batch; total DMA ~24 MiB ~ 70us, hidden under PE.
"""

import numpy as np

import concourse.mybir as mybir
import concourse.tile as tile
from concourse import bacc
from concourse.bass_utils import run_bass_kernel_spmd

N_CORES = 8
IN_F = 4096
OUT_F = 4096
P = 4
Q = 4
R = 512
CB = IN_F // Q        # 1024 input features per q block
OB = OUT_F // P       # 1024 output features per p block
N_TOK = 4 * 2048      # 8192 total tokens
N_CORE = N_TOK // N_CORES   # 1024 tokens per core

CHUNK = 512           # tokens per pipeline chunk
KT1 = CB // 128       # 8 contraction tiles per q in mm1
RT = R // 128         # 4 rank partition tiles
KB = 4                # k-tiles per x DMA batch
OG = 8                # o-groups per out DMA batch
OT = OB // 128        # 8 output-feature tiles per p

F32 = mybir.dt.float32
BF16 = mybir.dt.bfloat16
MULT = mybir.AluOpType.mult
ADD = mybir.AluOpType.add
IDENT = mybir.ActivationFunctionType.Identity

_cached_nc = None


def _build(n_core=N_CORE, chunk=CHUNK):
    nc = bacc.Bacc("TRN2", target_bir_lowering=False, debug=False,
                   enable_asserts=False)

    def din(name, shape, dtype):
        return nc.dram_tensor(name, shape, dtype, kind="ExternalInput").ap()

    xt = din("xt", [IN_F, n_core], BF16)
    ct = din("ct", [IN_F, R], BF16)
    bt = din("bt", [P * R, OB], BF16)
    dr = din("dr", [R, P * Q], F32)
    biasd = din("biasd", [OUT_F], F32)
    outT = nc.dram_tensor("outT", [OUT_F, n_core], BF16,
                          kind="ExternalOutput").ap()

    n_chunks = n_core // chunk

    with tile.TileContext(nc) as tc:
        with (
            tc.tile_pool(name="const", bufs=1) as cpool,
            tc.tile_pool(name="xp", bufs=4) as xpool,
            tc.tile_pool(name="ybp", bufs=16) as ybpool,
            tc.tile_pool(name="tp", bufs=6) as tpool,
            tc.tile_pool(name="zbp", bufs=2 * P * RT * n_chunks) as zbpool,
            tc.tile_pool(name="outp", bufs=4) as outpool,
            tc.tile_pool(name="yps", bufs=6, space="PSUM") as ypool,
            tc.tile_pool(name="ops", bufs=2, space="PSUM") as opool,
        ):
            # ct_sb[c_, q*8+k, r]: C^T tile rows c = (q*8+k)*128 + c_
            ct_sb = cpool.tile([128, IN_F // 128, R], BF16)
            ct3 = ct.rearrange("(t p) r -> p t r", p=128)
            # bt_sb[r_, p*4+rt, o]: B^T tile rows r = (p*4+rt)*128 + r_
            bt_sb = cpool.tile([128, (P * R) // 128, OB], BF16)
            bt3 = bt.rearrange("(t p) o -> p t o", p=128)
            # d_sb[r_, rt, p*4+q] = D[p, q, rt*128 + r_]
            d_sb = cpool.tile([128, RT, P * Q], F32)
            # bias_sb[o_, g] = bias[g*128 + o_]  (g = p*OT + ot)
            bias_sb = cpool.tile([128, OUT_F // 128], F32)

            zb = {}

            def emit_mm1(j):
                for q in range(Q):
                    ys = [
                        ypool.tile([128, chunk], F32, tag="y",
                                   name=f"y_{j}_{q}_{rt}")
                        for rt in range(RT)
                    ]
                    for kb in range(KT1 // KB):
                        base_t = q * KT1 + kb * KB
                        first = j == 0 and q == 0 and kb == 0
                        x_t = xpool.tile([128, KB, chunk], BF16, tag="x",
                                         name=f"x_{j}_{q}_{kb}")

                        def xdma(lo, hi):
                            nc.sync.dma_start(
                                x_t[:, lo:hi, :],
                                xt[(base_t + lo) * 128:(base_t + hi) * 128,
                                   j * chunk:(j + 1) * chunk]
                                .rearrange("(t p) n -> p t n", p=128))

                        def cdma(lo, hi):
                            hs = slice(base_t + lo, base_t + hi)
                            nc.sync.dma_start(ct_sb[:, hs, :], ct3[:, hs, :])

                        if first:
                            # k0's C then x ship alone so the first matmul
                            # waits on ~256 KiB, then interleaved k1 / k2-3
                            # pieces so compute overlaps the cold DMA stream
                            cdma(0, 1)
                            xdma(0, 1)
                            cdma(1, 2)
                            xdma(1, 2)
                            cdma(2, KB)
                            xdma(2, KB)
                        elif j == 0 and q < 2:
                            # half-batches while the DMA pipeline fills:
                            # delivery order matches PE consumption order
                            cdma(0, 2)
                            xdma(0, 2)
                            cdma(2, KB)
                            xdma(2, KB)
                        else:
                            if j == 0:
                                cdma(0, KB)
                            xdma(0, KB)
                        if j == 0 and q == 0 and kb == 1:
                            nc.sync.dma_start(
                                d_sb[:],
                                dr.rearrange("(t p) s -> p t s", p=128))
                            nc.sync.dma_start(
                                bias_sb[:],
                                biasd.rearrange("(t p) -> p t", p=128))
                        if j == n_chunks - 1:
                            # B prefetch in 512KiB pieces spread over the
                            # last mm1 chunk (8 slots): small enough not to
                            # starve the x stream, early enough for mm2 c0
                            idx = q * 2 + kb
                            bs = slice(idx * 2, idx * 2 + 2)
                            nc.sync.dma_start(bt_sb[:, bs, :], bt3[:, bs, :])
                        for kk in range(KB):
                            k = kb * KB + kk
                            for rt in range(RT):
                                nc.tensor.matmul(
                                    ys[rt][:],
                                    lhsT=ct_sb[:, base_t + kk,
                                               rt * 128:(rt + 1) * 128],
                                    rhs=x_t[:, kk, :],
                                    start=(k == 0), stop=(k == KT1 - 1))
                    # y -> SBUF bf16 on ACT, then the D-mix on DVE+Pool
                    ybs = []
                    for rt in range(RT):
                        yb_t = ybpool.tile([128, chunk], BF16, tag="yb",
                                           name=f"yb_{j}_{q}_{rt}")
                        nc.scalar.copy(yb_t[:], ys[rt][:])
                        ybs.append(yb_t)
                    # TensorScalarPtr is DVE-only on real HW (walrus rejects
                    # it on Pool), but TensorTensor runs on Pool too. bf16
                    # SBUF ops hit the DVE fast paths: tensor_scalar_mul at
                    # 4x, tensor_tensor add at 2x -- mul+add (520ns) beats
                    # the fused 1x stt (593ns). The q2 adds go to the
                    # otherwise-idle Pool engine so DVE stays under mm1's
                    # per-q cadence and its queue never backs up into mm2.
                    for rt in range(RT):
                        for p in range(P):
                            col = p * Q + q
                            dcol = d_sb[:, rt, col:col + 1]
                            if q == 0:
                                zt = zbpool.tile([128, chunk], BF16, tag="zb",
                                                 name=f"zb_{j}_{p}_{rt}")
                                zb[(j, p, rt)] = zt
                                nc.vector.tensor_scalar_mul(
                                    zt[:], ybs[rt][:], dcol)
                            else:
                                zt = zb[(j, p, rt)]
                                tt = tpool.tile([128, chunk], BF16, tag="t",
                                                name=f"t_{j}_{q}_{p}_{rt}")
                                nc.vector.tensor_scalar_mul(
                                    tt[:], ybs[rt][:], dcol)
                                eng = nc.gpsimd if q == 2 else nc.vector
                                eng.tensor_tensor(
                                    zt[:], tt[:], zt[:], op=ADD)

            def emit_mm2(j):
                ob_t = None
                for p in range(P):
                    for ot in range(OT):
                        g = p * OT + ot
                        ops = opool.tile([128, chunk], F32, tag="o",
                                         name=f"o_{j}_{g}")
                        for rt in range(RT):
                            nc.tensor.matmul(
                                ops[:],
                                lhsT=bt_sb[:, p * RT + rt,
                                           ot * 128:(ot + 1) * 128],
                                rhs=zb[(j, p, rt)][:],
                                start=(rt == 0), stop=(rt == RT - 1))
                        # batch OG o-groups per out DMA: per-DMA HWDGE
                        # descriptor-gen is a fixed ~625ns on a single
                        # shared device, so per-group DMAs can't keep up
                        # with the 873ns group cadence. The kernel's last
                        # few groups ship in smaller pieces so the final
                        # DMA is small and starts right after its drain.
                        NG = P * OT
                        last = j == n_chunks - 1
                        og = OG
                        if g % og == 0:
                            ob_t = outpool.tile([128, og, chunk], BF16,
                                                tag="ob", name=f"ob_{j}_{g}")
                        dst = ob_t[:, g % og, :]
                        if last and g >= NG - 2:
                            # final two groups: drain column-halves on ACT
                            # and DVE in parallel, each half shipped as its
                            # own 128KiB DMA -- shortens the end-of-kernel
                            # drain+DMA tail
                            h = chunk // 2
                            nc.scalar.activation(
                                dst[:, 0:h], ops[:, 0:h], IDENT,
                                bias=bias_sb[:, g:g + 1], scale=1.0)
                            nc.vector.tensor_scalar_add(
                                dst[:, h:chunk], ops[:, h:chunk],
                                bias_sb[:, g:g + 1])
                            nc.sync.dma_start(
                                outT[g * 128:(g + 1) * 128,
                                     j * chunk:j * chunk + h],
                                dst[:, 0:h])
                            nc.sync.dma_start(
                                outT[g * 128:(g + 1) * 128,
                                     j * chunk + h:(j + 1) * chunk],
                                dst[:, h:chunk])
                            continue
                        # fused PSUM drain + per-partition bias add.
                        # Alternate ACT/DVE so consecutive o-groups drain in
                        # parallel: with only 2 PSUM o-banks, a single
                        # engine's ~950ns drain latency exceeds the 873ns
                        # group time and stalls PE ~120ns/group. In mm2 c0
                        # DVE is still draining chunk 1's mix queue, so all
                        # of c0 stays on ACT.
                        use_dve = (g % 2 == 1) and last
                        if use_dve:
                            nc.vector.tensor_scalar_add(
                                dst, ops[:], bias_sb[:, g:g + 1])
                        else:
                            nc.scalar.activation(
                                dst, ops[:], IDENT,
                                bias=bias_sb[:, g:g + 1], scale=1.0)
                        if last and NG - 4 <= g < NG - 2:
                            # groups 28/29: per-group DMAs
                            nc.sync.dma_start(
                                outT[g * 128:(g + 1) * 128,
                                     j * chunk:(j + 1) * chunk],
                                dst)
                        elif g % og == og - 1 or (last and g == NG - 5):
                            # flush accumulated slots (a full batch, or the
                            # partial batch ending right before the tail)
                            nslot = g % og + 1
                            gb = g - (g % og)
                            nc.sync.dma_start(
                                outT[gb * 128:(g + 1) * 128,
                                     j * chunk:(j + 1) * chunk]
                                .rearrange("(t p) n -> p t n", p=128),
                                ob_t[:, 0:nslot, :])

            for j in range(n_chunks):
                emit_mm1(j)
            for j in range(n_chunks):
                emit_mm2(j)

    nc.compile()
    return nc


def _prep_in_maps(x, B, C, D, bias):
    import ml_dtypes
    x2 = np.asarray(x, dtype=np.float32).reshape(N_TOK, IN_F)
    CT = np.ascontiguousarray(
        np.asarray(C, dtype=np.float32).transpose(0, 2, 1).reshape(IN_F, R)
    ).astype(ml_dtypes.bfloat16)
    BT = np.ascontiguousarray(
        np.asarray(B, dtype=np.float32).transpose(0, 2, 1).reshape(P * R, OB)
    ).astype(ml_dtypes.bfloat16)
    DR = np.ascontiguousarray(
        np.asarray(D, dtype=np.float32).transpose(2, 0, 1).reshape(R, P * Q))
    BI = np.ascontiguousarray(np.asarray(bias, dtype=np.float32))

    in_maps = []
    for c in range(N_CORES):
        xtc = np.ascontiguousarray(
            x2[c * N_CORE:(c + 1) * N_CORE].T).astype(ml_dtypes.bfloat16)
        in_maps.append({
            "xt": xtc, "ct": CT, "bt": BT, "dr": DR, "biasd": BI,
        })
    return in_maps


def _run(in_maps, trace=False):
    global _cached_nc
    if _cached_nc is None:
        _cached_nc = _build()
    import time
    for attempt in range(3):
        try:
            return run_bass_kernel_spmd(
                _cached_nc, in_maps, list(range(N_CORES)), trace=trace)
        except Exception:
            # transient device errors (e.g. NRT_EXEC_UNIT_UNRECOVERABLE
            # from a previously wedged core) usually clear on retry
            if attempt == 2:
                raise
            time.sleep(5.0 * (attempt + 1))


def kernel(x, B, C, D, bias):
    lead = np.asarray(x).shape[:-1]
    res = _run(_prep_in_maps(x, B, C, D, bias))
    outs = [
        np.asarray(res.results[c]["outT"]).astype(np.float32).T
        for c in range(N_CORES)
    ]
    return np.concatenate(outs, axis=0).reshape(*lead, OUT_F)
